# revision 11
# baseline (speedup 1.0000x reference)
"""DirectVoxGO forward kernel for 8 Trainium2 NeuronCores (Bass/Tile).

Data-parallel over rays (1024 rays x 256 samples per core). Host packs an
xy-duplicated voxel record table (159*159*160 records of 52 bf16 =
[4 xy-corner voxels x 13ch]); each sample's 8 trilinear corners = one
indirect-DMA gather of 2 consecutive records (z, z+1). On-device:
trilinear lerp (DVE), density->alpha via exp series (ACT+DVE), per-ray
transmittance cumsum (PE matmuls), per-tile PE transpose to
feature-major, 3-layer MLP in bf16 (PE), weighted composite (DVE).
"""
import sys
import numpy as np

sys.path.insert(0, "/opt/trn_rl_repo")

import ml_dtypes  # noqa: E402
from contextlib import ExitStack  # noqa: E402

import concourse.bass as bass  # noqa: E402
import concourse.tile as tile  # noqa: E402
from concourse import mybir, bacc  # noqa: E402
from concourse.bass_utils import run_bass_kernel_spmd  # noqa: E402

G = 160
CF = 12
CH = 13              # density + 12 feat channels per voxel
NR, NS = 8192, 256
NCORES = 8
RAYS_PER_CORE = NR // NCORES          # 1024
S_PER_CORE = RAYS_PER_CORE * NS       # 262144
BLK = 2048                            # samples per block (8 rays)
NT = BLK // 128                       # 16 tiles of 128 samples
NBLK_FULL = S_PER_CORE // BLK         # 128
SB = 8                                # blocks per superblock
NRECT = (G - 1) * (G - 1) * G
REC = 4 * CH                          # 52 elems per record

ALPHA_INIT = 1e-6
ACT_SHIFT = float(np.log(1.0 / (1.0 - ALPHA_INIT) - 1.0))
EPS = 1e-10
BF16 = ml_dtypes.bfloat16

_cache = {}


def _ap(src, extra_dims=None, dims=None, offset_add=0):
    """Build a raw AP from an existing AP, appending/replacing dims."""
    a = src
    new = list(a.ap) + list(extra_dims or []) if dims is None else list(dims)
    return bass.AP(tensor=a.tensor, offset=a.offset + offset_add, ap=new)


def build_table(density_grid, k0):
    g13 = np.concatenate([np.asarray(k0).astype(BF16),
                          np.asarray(density_grid).astype(BF16)], axis=-1)
    A = g13[:-1, :-1]
    B = g13[:-1, 1:]
    C = g13[1:, :-1]
    D = g13[1:, 1:]
    tab = np.concatenate([A, B, C, D], axis=-1)  # [159,159,160,52]
    return np.ascontiguousarray(tab.reshape(NRECT, REC))


def build_nc(nblk):
    nc = bacc.Bacc("TRN2", target_bir_lowering=False, debug=False, num_devices=NCORES)
    f32, bf16, i32 = mybir.dt.float32, mybir.dt.bfloat16, mybir.dt.int32
    AF = mybir.ActivationFunctionType
    OP = mybir.AluOpType

    t_tab = nc.dram_tensor("tab", [NRECT, REC], bf16, kind="ExternalInput")
    t_rp = nc.dram_tensor("rp", [nblk, 128, NT, 3], f32, kind="ExternalInput")
    t_vd = nc.dram_tensor("vd", [128, 8, 3], f32, kind="ExternalInput")
    t_w0a = nc.dram_tensor("w0a", [CF, 128], bf16, kind="ExternalInput")
    t_w0b = nc.dram_tensor("w0b", [27, 128], bf16, kind="ExternalInput")
    t_w1 = nc.dram_tensor("w1", [128, 128], bf16, kind="ExternalInput")
    t_w2 = nc.dram_tensor("w2", [128, 3], bf16, kind="ExternalInput")
    t_b0 = nc.dram_tensor("b0", [128, 1], f32, kind="ExternalInput")
    t_b1 = nc.dram_tensor("b1", [128, 1], f32, kind="ExternalInput")
    t_b2 = nc.dram_tensor("b2", [3, 1], f32, kind="ExternalInput")
    t_ustr = nc.dram_tensor("ustr", [128, 128], f32, kind="ExternalInput")
    t_ones = nc.dram_tensor("onesA", [128, 128], f32, kind="ExternalInput")
    t_iden = nc.dram_tensor("iden", [128, 128], bf16, kind="ExternalInput")
    t_out = nc.dram_tensor("out", [3, nblk * 8], f32, kind="ExternalOutput")

    nrays = nblk * 8

    with ExitStack() as ctx:
        tc = ctx.enter_context(tile.TileContext(nc))
        singles = ctx.enter_context(tc.tile_pool(name="singles", bufs=1))
        rp_pool = ctx.enter_context(tc.tile_pool(name="rpp", bufs=2))
        sbp = ctx.enter_context(tc.tile_pool(name="sbp", bufs=2))
        gpool = ctx.enter_context(tc.tile_pool(name="gp", bufs=3))
        lpool = ctx.enter_context(tc.tile_pool(name="lp", bufs=2))
        spool = ctx.enter_context(tc.tile_pool(name="spp", bufs=2))
        xpool = ctx.enter_context(tc.tile_pool(name="xp", bufs=2))
        hpool = ctx.enter_context(tc.tile_pool(name="hp", bufs=2))
        ps_mlp = ctx.enter_context(tc.tile_pool(name="psm", bufs=1, space="PSUM"))
        ps_tr = ctx.enter_context(tc.tile_pool(name="pst", bufs=2, space="PSUM"))
        ps_cs = ctx.enter_context(tc.tile_pool(name="psc", bufs=1, space="PSUM"))

        # ---- constants ----
        w0a = singles.tile([CF, 128], bf16)
        nc.sync.dma_start(out=w0a[:], in_=t_w0a[:])
        w0b = singles.tile([27, 128], bf16)
        nc.sync.dma_start(out=w0b[:], in_=t_w0b[:])
        w1 = singles.tile([128, 128], bf16)
        nc.sync.dma_start(out=w1[:], in_=t_w1[:])
        w2 = singles.tile([128, 3], bf16)
        nc.sync.dma_start(out=w2[:], in_=t_w2[:])
        b0 = singles.tile([128, 1], f32)
        nc.sync.dma_start(out=b0[:], in_=t_b0[:])
        b1 = singles.tile([128, 1], f32)
        nc.sync.dma_start(out=b1[:], in_=t_b1[:])
        b2 = singles.tile([3, 1], f32)
        nc.sync.dma_start(out=b2[:], in_=t_b2[:])
        ustr = singles.tile([128, 128], f32)
        nc.sync.dma_start(out=ustr[:], in_=t_ustr[:])
        onesA = singles.tile([128, 128], f32)
        nc.sync.dma_start(out=onesA[:], in_=t_ones[:])
        iden = singles.tile([128, 128], bf16)
        nc.sync.dma_start(out=iden[:], in_=t_iden[:])
        c_shift = singles.tile([128, 1], f32)
        nc.vector.memset(c_shift[:], ACT_SHIFT)
        c_hpi = singles.tile([128, 1], f32)
        nc.vector.memset(c_hpi[:], float(np.pi / 2))
        negb2 = singles.tile([3, 1], f32)
        nc.vector.tensor_scalar(out=negb2[:], in0=b2[:], scalar1=-1.0, scalar2=None, op0=OP.mult)

        # ---- prologue: view embedding -> vembT [27, nrays] bf16 ----
        vd = singles.tile([128, 8, 3], f32)
        nc.sync.dma_start(out=vd[:], in_=t_vd[:])
        sq = singles.tile([128, 8, 3], f32)
        nc.vector.tensor_tensor(out=sq[:], in0=vd[:], in1=vd[:], op=OP.mult)
        ss = singles.tile([128, 8], f32)
        nc.vector.reduce_sum(out=ss[:], in_=sq[:], axis=mybir.AxisListType.X)
        nrm = singles.tile([128, 8], f32)
        nc.scalar.activation(out=nrm[:], in_=ss[:], func=AF.Sqrt)
        nc.vector.tensor_scalar(out=nrm[:], in0=nrm[:], scalar1=EPS, scalar2=None, op0=OP.add)
        rinv = singles.tile([128, 8], f32)
        nc.vector.reciprocal(out=rinv[:], in_=nrm[:])
        dn = singles.tile([128, 8, 3], f32)
        nc.vector.tensor_tensor(out=dn[:], in0=vd[:], in1=_ap(rinv[:], [[0, 3]]), op=OP.mult)
        ang = singles.tile([128, 8, 12], f32)
        for k in range(4):
            dstk = _ap(ang[:], dims=[ang[:].ap[0], [12, 8], [4, 3]], offset_add=k)
            nc.vector.tensor_scalar(out=dstk, in0=dn[:], scalar1=float(2 ** k),
                                    scalar2=None, op0=OP.mult)
        V = singles.tile([128, 8, 27], f32)
        nc.vector.tensor_copy(V[:, :, 0:3], dn[:])
        nc.scalar.activation(out=V[:, :, 3:15], in_=ang[:], func=AF.Sin)
        nc.scalar.activation(out=V[:, :, 15:27], in_=ang[:], func=AF.Sin, bias=c_hpi[:])
        vembT = singles.tile([27, 1024], bf16)
        for j in range(8):
            Vj = singles.tile([128, 27], bf16, tag=f"vjc{j}", name=f"vjc{j}")
            nc.vector.tensor_copy(Vj[:], V[:, j, :])
            pv = ps_tr.tile([27, 128], bf16, tag="pt")
            nc.tensor.transpose(out=pv[:], in_=Vj[:], identity=iden[:])
            nc.vector.tensor_copy(vembT[:, j * 128:(j + 1) * 128], pv[:])

        outacc = singles.tile([3, nrays], f32)
        blast = singles.tile([3, nrays], f32)

        # ---- main loop ----
        nsb = (nblk + SB - 1) // SB
        for isb in range(nsb):
            base = isb * SB
            nb = min(SB, nblk - base)
            rp = rp_pool.tile([128, SB, NT, 3], f32, tag="rp")
            nc.sync.dma_start(out=rp[:, 0:nb],
                              in_=t_rp[base:base + nb].rearrange("b p t c -> p b t c"))
            u = sbp.tile([128, SB, NT, 3], f32, tag="u")
            nc.vector.tensor_scalar(out=u[:, 0:nb], in0=rp[:, 0:nb], scalar1=float(G - 1),
                                    scalar2=0.0, op0=OP.mult, op1=OP.max)
            nc.vector.tensor_scalar(out=u[:, 0:nb], in0=u[:, 0:nb], scalar1=float(G - 1),
                                    scalar2=None, op0=OP.min)
            i0 = sbp.tile([128, SB, NT, 3], i32, tag="i0")
            nc.vector.tensor_copy(i0[:, 0:nb], u[:, 0:nb])
            i0f = sbp.tile([128, SB, NT, 3], f32, tag="i0f")
            nc.vector.tensor_copy(i0f[:, 0:nb], i0[:, 0:nb])
            # exact floor fixup: if i0f > u: i0f -= 1  (handles any cast rounding)
            gt = sbp.tile([128, SB, NT, 3], f32, tag="gt")
            nc.vector.tensor_tensor(out=gt[:, 0:nb], in0=i0f[:, 0:nb], in1=u[:, 0:nb], op=OP.is_gt)
            nc.vector.tensor_tensor(out=i0f[:, 0:nb], in0=i0f[:, 0:nb], in1=gt[:, 0:nb], op=OP.subtract)
            nc.vector.tensor_scalar(out=i0f[:, 0:nb], in0=i0f[:, 0:nb], scalar1=float(G - 2),
                                    scalar2=None, op0=OP.min)
            fr = sbp.tile([128, SB, NT, 3], f32, tag="fr")
            nc.vector.tensor_tensor(out=fr[:, 0:nb], in0=u[:, 0:nb], in1=i0f[:, 0:nb], op=OP.subtract)
            frb = sbp.tile([128, SB, NT, 3], bf16, tag="frb")
            nc.vector.tensor_copy(frb[:, 0:nb], fr[:, 0:nb])
            nc.vector.tensor_copy(i0[:, 0:nb], i0f[:, 0:nb])  # exact ints, any rounding ok
            idxt = sbp.tile([128, SB, NT], i32, tag="idxt")
            nc.vector.tensor_scalar(out=idxt[:, 0:nb], in0=i0[:, 0:nb, :, 0],
                                    scalar1=(G - 1) * G, scalar2=None, op0=OP.mult)
            tmpy = sbp.tile([128, SB, NT], i32, tag="tmpy")
            nc.vector.tensor_scalar(out=tmpy[:, 0:nb], in0=i0[:, 0:nb, :, 1],
                                    scalar1=G, scalar2=None, op0=OP.mult)
            nc.vector.tensor_tensor(out=idxt[:, 0:nb], in0=idxt[:, 0:nb], in1=tmpy[:, 0:nb], op=OP.add)
            nc.vector.tensor_tensor(out=idxt[:, 0:nb], in0=idxt[:, 0:nb], in1=i0[:, 0:nb, :, 2], op=OP.add)

            for ib in range(nb):
                b = base + ib
                Gt = gpool.tile([128, NT, 2 * REC], bf16, tag="G")
                for t in range(NT):
                    nc.gpsimd.indirect_dma_start(
                        out=Gt[:, t, :], out_offset=None, in_=t_tab[:],
                        in_offset=bass.IndirectOffsetOnAxis(ap=idxt[:, ib, t:t + 1], axis=0))

                fx = frb[:, ib, :, 0]
                fy = frb[:, ib, :, 1]
                fz = frb[:, ib, :, 2]
                # z-lerp
                zl = lpool.tile([128, NT, REC], bf16, tag="zl")
                nc.vector.tensor_tensor(out=zl[:], in0=Gt[:, :, REC:2 * REC],
                                        in1=Gt[:, :, 0:REC], op=OP.subtract)
                nc.vector.tensor_tensor(out=zl[:], in0=zl[:], in1=_ap(fz, [[0, REC]]), op=OP.mult)
                nc.vector.tensor_tensor(out=zl[:], in0=zl[:], in1=Gt[:, :, 0:REC], op=OP.add)
                # y-lerp on (0:13 | 13:26) and (26:39 | 39:52)
                yl = lpool.tile([128, NT, 2, CH], bf16, tag="yl")
                z0 = zl[:]
                zy0 = _ap(z0, dims=[z0.ap[0], z0.ap[1], [2 * CH, 2], [1, CH]])
                zy1 = _ap(z0, dims=[z0.ap[0], z0.ap[1], [2 * CH, 2], [1, CH]], offset_add=CH)
                fyb = _ap(fy, [[0, 2], [0, CH]])
                nc.vector.tensor_tensor(out=yl[:], in0=zy1, in1=zy0, op=OP.subtract)
                nc.vector.tensor_tensor(out=yl[:], in0=yl[:], in1=fyb, op=OP.mult)
                nc.vector.tensor_tensor(out=yl[:], in0=yl[:], in1=zy0, op=OP.add)
                # x-lerp -> L[:,:,0:13]
                L = lpool.tile([128, NT, 16], bf16, tag="L")
                nc.vector.tensor_tensor(out=L[:, :, 0:CH], in0=yl[:, :, 1, :], in1=yl[:, :, 0, :], op=OP.subtract)
                nc.vector.tensor_tensor(out=L[:, :, 0:CH], in0=L[:, :, 0:CH], in1=_ap(fx, [[0, CH]]), op=OP.mult)
                nc.vector.tensor_tensor(out=L[:, :, 0:CH], in0=L[:, :, 0:CH], in1=yl[:, :, 0, :], op=OP.add)

                # density -> u=exp(d+shift); sp=u(1-u/2); alpha=0.5u(1-0.75u)
                uu = spool.tile([128, NT], f32, tag="uu")
                nc.scalar.activation(out=uu[:], in_=L[:, :, 12], func=AF.Exp, bias=c_shift[:])
                sp_t = spool.tile([128, NT], f32, tag="sp_t")
                nc.vector.tensor_scalar(out=sp_t[:], in0=uu[:], scalar1=-0.5, scalar2=1.0,
                                        op0=OP.mult, op1=OP.add)
                nc.vector.tensor_tensor(out=sp_t[:], in0=sp_t[:], in1=uu[:], op=OP.mult)
                alpha = spool.tile([128, NT], f32, tag="alpha")
                nc.vector.tensor_scalar(out=alpha[:], in0=uu[:], scalar1=-0.375, scalar2=0.5,
                                        op0=OP.mult, op1=OP.add)
                nc.vector.tensor_tensor(out=alpha[:], in0=alpha[:], in1=uu[:], op=OP.mult)

                # cumsum: S_excl(p,2r)=sum_{q<p} sp[q,2r]; odd cols += colsum(even)
                pscomb = ps_cs.tile([128, 2 * NT], f32, tag="cs")
                psS = pscomb[:, 0:NT]
                psC = pscomb[:, NT:2 * NT]
                nc.tensor.matmul(out=psS, lhsT=ustr[:], rhs=sp_t[:], start=True, stop=True)
                nc.tensor.matmul(out=psC, lhsT=onesA[:], rhs=sp_t[:], start=True, stop=True)
                csb = spool.tile([128, NT], f32, tag="csb")
                nc.vector.tensor_copy(csb[:], psC)
                sex = spool.tile([128, NT], f32, tag="sex")
                pse = _ap(psS, dims=[psS.ap[0], [2, NT // 2]])
                pso = _ap(psS, dims=[psS.ap[0], [2, NT // 2]], offset_add=1)
                cse = _ap(csb[:], dims=[csb[:].ap[0], [2, NT // 2]])
                sxe = _ap(sex[:], dims=[sex[:].ap[0], [2, NT // 2]])
                sxo = _ap(sex[:], dims=[sex[:].ap[0], [2, NT // 2]], offset_add=1)
                nc.vector.tensor_copy(sxe, pse)
                nc.vector.tensor_tensor(out=sxo, in0=pso, in1=cse, op=OP.add)
                A_t = spool.tile([128, NT], f32, tag="A_t")
                nc.scalar.activation(out=A_t[:], in_=sex[:], func=AF.Exp, scale=-0.5)
                nc.vector.tensor_tensor(out=L[:, :, 13], in0=alpha[:], in1=A_t[:], op=OP.mult)
                nc.vector.tensor_tensor(out=L[:, :, 14], in0=alpha[:], in1=A_t[:], op=OP.mult)
                nc.vector.tensor_tensor(out=L[:, :, 15], in0=alpha[:], in1=A_t[:], op=OP.mult)
                rtot = spool.tile([3, 8], f32, tag="rtot")
                cb3 = csb[0:3, :]
                ce3 = _ap(cb3, dims=[[cb3.ap[0][0], 3], [2, NT // 2]])
                co3 = _ap(cb3, dims=[[cb3.ap[0][0], 3], [2, NT // 2]], offset_add=1)
                nc.vector.tensor_tensor(out=rtot[:], in0=ce3, in1=co3, op=OP.add)
                nc.scalar.activation(out=blast[:, b * 8:(b + 1) * 8], in_=rtot[:], func=AF.Exp, scale=-0.5)

                # transpose -> XT [16, BLK]
                XT = xpool.tile([16, BLK], bf16, tag="XT")
                wT3 = xpool.tile([3, BLK], bf16, tag="wT3")
                for t in range(NT):
                    pt = ps_tr.tile([16, 128], bf16, tag="pt")
                    nc.tensor.transpose(out=pt[:], in_=L[:, t, :], identity=iden[:])
                    nc.vector.tensor_copy(XT[:, t * 128:(t + 1) * 128], pt[:])
                    pw = ps_tr.tile([3, 128], bf16, tag="pw")
                    nc.tensor.transpose(out=pw[:], in_=L[:, t, 13:16], identity=iden[:])
                    nc.vector.tensor_copy(wT3[:, t * 128:(t + 1) * 128], pw[:])

                vb = xpool.tile([27, BLK], bf16, tag="vb")
                vsl = vembT[:, b * 8:(b + 1) * 8]
                nc.vector.tensor_copy(vb[:].rearrange("p (r k) -> p r k", r=8),
                                      _ap(vsl, [[0, 256]]))

                for c in range(4):
                    cs = slice(c * 512, (c + 1) * 512)
                    p0 = ps_mlp.tile([128, 512], f32, tag="p0")
                    nc.tensor.matmul(out=p0[:], lhsT=w0a[:], rhs=XT[0:CF, cs], start=True, stop=False)
                    nc.tensor.matmul(out=p0[:], lhsT=w0b[:], rhs=vb[:, cs], start=False, stop=True)
                    h0 = hpool.tile([128, 512], bf16, tag="h0")
                    nc.scalar.activation(out=h0[:], in_=p0[:], func=AF.Relu, bias=b0[:])
                    p1 = ps_mlp.tile([128, 512], f32, tag="p1")
                    nc.tensor.matmul(out=p1[:], lhsT=w1[:], rhs=h0[:], start=True, stop=True)
                    h1 = hpool.tile([128, 512], bf16, tag="h1")
                    nc.scalar.activation(out=h1[:], in_=p1[:], func=AF.Relu, bias=b1[:])
                    p2 = ps_mlp.tile([3, 512], f32, tag="p2")
                    nc.tensor.matmul(out=p2[:], lhsT=w2[:], rhs=h1[:], start=True, stop=True)
                    ex = hpool.tile([3, 512], f32, tag="ex")
                    nc.scalar.activation(out=ex[:], in_=p2[:], func=AF.Exp, scale=-1.0, bias=negb2[:])
                    nc.vector.tensor_scalar(out=ex[:], in0=ex[:], scalar1=1.0, scalar2=None, op0=OP.add)
                    rgb = hpool.tile([3, 512], f32, tag="rgb")
                    nc.vector.reciprocal(out=rgb[:], in_=ex[:])
                    nc.vector.tensor_tensor(out=rgb[:], in0=rgb[:],
                                            in1=wT3[:, cs], op=OP.mult)
                    rr = rgb[:]
                    rseg = _ap(rr, dims=[rr.ap[0], [256, 2], [1, 256]])
                    nc.vector.reduce_sum(out=outacc[:, b * 8 + c * 2: b * 8 + c * 2 + 2],
                                         in_=rseg, axis=mybir.AxisListType.X)

        nc.vector.tensor_tensor(out=outacc[:], in0=outacc[:], in1=blast[:], op=OP.add)
        nc.sync.dma_start(out=t_out[:], in_=outacc[:])

    nc.compile()
    return nc


def _host_prep(inputs, nblk):
    table = build_table(inputs["density_grid"], inputs["k0"])
    w0 = np.asarray(inputs["w0"])
    consts = {
        "tab": table,
        "w0a": np.ascontiguousarray(w0[0:CF]).astype(BF16),
        "w0b": np.ascontiguousarray(w0[CF:CF + 27]).astype(BF16),
        "w1": np.asarray(inputs["w1"]).astype(BF16),
        "w2": np.asarray(inputs["w2"]).astype(BF16),
        "b0": np.asarray(inputs["b0"]).astype(np.float32).reshape(128, 1),
        "b1": np.asarray(inputs["b1"]).astype(np.float32).reshape(128, 1),
        "b2": np.asarray(inputs["b2"]).astype(np.float32).reshape(3, 1),
        "ustr": np.triu(np.ones((128, 128), np.float32), 1),  # [q,p]=1 if q<p
        "onesA": np.ones((128, 128), np.float32),
        "iden": np.eye(128, dtype=np.float32).astype(BF16),
    }
    rp_all = np.asarray(inputs["ray_pts"], np.float32)
    vd_all = np.asarray(inputs["viewdirs"], np.float32)
    in_maps = []
    for c in range(NCORES):
        rp = rp_all[c * RAYS_PER_CORE:(c + 1) * RAYS_PER_CORE].reshape(-1, 3)
        rp = rp[: nblk * BLK]
        rp = rp.reshape(nblk, NT, 128, 3).transpose(0, 2, 1, 3)  # sample = t*128+p
        vd = vd_all[c * RAYS_PER_CORE:(c + 1) * RAYS_PER_CORE]
        vd = vd.reshape(8, 128, 3).transpose(1, 0, 2)            # ray = j*128+p
        in_maps.append(dict(consts, rp=np.ascontiguousarray(rp), vd=np.ascontiguousarray(vd)))
    return in_maps


def run(inputs, nblk=NBLK_FULL, profile=False):
    if nblk not in _cache:
        _cache[nblk] = build_nc(nblk)
    nc = _cache[nblk]
    in_maps = _host_prep(inputs, nblk)
    if profile:
        from bench_lib import run_and_profile
        results, exec_ns = run_and_profile(nc, in_maps, profile=True)
        return results, exec_ns
    res = run_bass_kernel_spmd(nc, in_maps, core_ids=list(range(NCORES)))
    return res.results, None


def kernel(**inputs):
    results, _ = run(inputs, NBLK_FULL)
    out = np.concatenate([r["out"].T for r in results], axis=0)
    return out.astype(np.float32)


# revision 12
# speedup vs baseline: 1.3908x; 1.3908x over previous
"""DirectVoxGO forward kernel for 8 Trainium2 NeuronCores (Bass/Tile).

Data-parallel over rays (1024 rays x 256 samples per core). Host packs an
xy-duplicated voxel record table (159*159*160 records of 52 bf16 =
[4 xy-corner voxels x 13ch]); each sample's 8 trilinear corners = one
indirect-DMA gather of 2 consecutive records (z, z+1). On-device:
trilinear lerp (DVE), density->alpha via exp series (ACT+DVE), per-ray
transmittance cumsum (PE matmuls), per-tile PE transpose to
feature-major, 3-layer MLP in bf16 (PE), weighted composite (DVE).
"""
import sys
import numpy as np

sys.path.insert(0, "/opt/trn_rl_repo")

import ml_dtypes  # noqa: E402
from contextlib import ExitStack  # noqa: E402

import concourse.bass as bass  # noqa: E402
import concourse.tile as tile  # noqa: E402
from concourse import mybir, bacc  # noqa: E402
from concourse.bass_utils import run_bass_kernel_spmd  # noqa: E402

G = 160
CF = 12
CH = 13              # density + 12 feat channels per voxel
NR, NS = 8192, 256
NCORES = 8
RAYS_PER_CORE = NR // NCORES          # 1024
S_PER_CORE = RAYS_PER_CORE * NS       # 262144
BLK = 2048                            # samples per block (8 rays)
NT = BLK // 128                       # 16 tiles of 128 samples
NBLK_FULL = S_PER_CORE // BLK         # 128
SB = 8                                # blocks per superblock
NRECT = (G - 1) * (G - 1) * G
REC = 4 * CH                          # 52 elems per record

ALPHA_INIT = 1e-6
ACT_SHIFT = float(np.log(1.0 / (1.0 - ALPHA_INIT) - 1.0))
EPS = 1e-10
BF16 = ml_dtypes.bfloat16

_cache = {}


def _ap(src, extra_dims=None, dims=None, offset_add=0):
    """Build a raw AP from an existing AP, appending/replacing dims."""
    a = src
    new = list(a.ap) + list(extra_dims or []) if dims is None else list(dims)
    return bass.AP(tensor=a.tensor, offset=a.offset + offset_add, ap=new)


def build_table(density_grid, k0):
    g13 = np.concatenate([np.asarray(k0).astype(BF16),
                          np.asarray(density_grid).astype(BF16)], axis=-1)
    A = g13[:-1, :-1]
    B = g13[:-1, 1:]
    C = g13[1:, :-1]
    D = g13[1:, 1:]
    tab = np.concatenate([A, B, C, D], axis=-1)  # [159,159,160,52]
    return np.ascontiguousarray(tab.reshape(NRECT, REC))


def build_nc(nblk):
    nc = bacc.Bacc("TRN2", target_bir_lowering=False, debug=False, num_devices=NCORES)
    f32, bf16, i32 = mybir.dt.float32, mybir.dt.bfloat16, mybir.dt.int32
    AF = mybir.ActivationFunctionType
    OP = mybir.AluOpType

    t_tab = nc.dram_tensor("tab", [NRECT, REC], bf16, kind="ExternalInput")
    t_rp = nc.dram_tensor("rp", [nblk, 128, NT, 3], f32, kind="ExternalInput")
    t_vd = nc.dram_tensor("vd", [128, 8, 3], f32, kind="ExternalInput")
    t_w0a = nc.dram_tensor("w0a", [CF, 128], bf16, kind="ExternalInput")
    t_w0b = nc.dram_tensor("w0b", [27, 128], bf16, kind="ExternalInput")
    t_w1 = nc.dram_tensor("w1", [128, 128], bf16, kind="ExternalInput")
    t_w2 = nc.dram_tensor("w2", [128, 3], bf16, kind="ExternalInput")
    t_b0 = nc.dram_tensor("b0", [128, 1], f32, kind="ExternalInput")
    t_b1 = nc.dram_tensor("b1", [128, 1], f32, kind="ExternalInput")
    t_b2 = nc.dram_tensor("b2", [3, 1], f32, kind="ExternalInput")
    t_ustr = nc.dram_tensor("ustr", [128, 128], f32, kind="ExternalInput")
    t_ones = nc.dram_tensor("onesA", [128, 128], f32, kind="ExternalInput")
    t_iden = nc.dram_tensor("iden", [128, 128], bf16, kind="ExternalInput")
    t_out = nc.dram_tensor("out", [3, nblk * 8], f32, kind="ExternalOutput")

    nrays = nblk * 8

    with ExitStack() as ctx:
        tc = ctx.enter_context(tile.TileContext(nc))
        singles = ctx.enter_context(tc.tile_pool(name="singles", bufs=1))
        rp_pool = ctx.enter_context(tc.tile_pool(name="rpp", bufs=2))
        sbp = ctx.enter_context(tc.tile_pool(name="sbp", bufs=2))
        gpool = ctx.enter_context(tc.tile_pool(name="gp", bufs=3))
        lpool = ctx.enter_context(tc.tile_pool(name="lp", bufs=2))
        spool = ctx.enter_context(tc.tile_pool(name="spp", bufs=2))
        xpool = ctx.enter_context(tc.tile_pool(name="xp", bufs=2))
        hpool = ctx.enter_context(tc.tile_pool(name="hp", bufs=2))
        ps_mlp = ctx.enter_context(tc.tile_pool(name="psm", bufs=1, space="PSUM"))
        ps_tr = ctx.enter_context(tc.tile_pool(name="pst", bufs=2, space="PSUM"))
        ps_cs = ctx.enter_context(tc.tile_pool(name="psc", bufs=1, space="PSUM"))

        # ---- constants ----
        w0a = singles.tile([CF, 128], bf16)
        nc.sync.dma_start(out=w0a[:], in_=t_w0a[:])
        w0b = singles.tile([27, 128], bf16)
        nc.sync.dma_start(out=w0b[:], in_=t_w0b[:])
        w1 = singles.tile([128, 128], bf16)
        nc.sync.dma_start(out=w1[:], in_=t_w1[:])
        w2 = singles.tile([128, 3], bf16)
        nc.sync.dma_start(out=w2[:], in_=t_w2[:])
        b0 = singles.tile([128, 1], f32)
        nc.sync.dma_start(out=b0[:], in_=t_b0[:])
        b1 = singles.tile([128, 1], f32)
        nc.sync.dma_start(out=b1[:], in_=t_b1[:])
        b2 = singles.tile([3, 1], f32)
        nc.sync.dma_start(out=b2[:], in_=t_b2[:])
        ustr = singles.tile([128, 128], f32)
        nc.sync.dma_start(out=ustr[:], in_=t_ustr[:])
        onesA = singles.tile([128, 128], f32)
        nc.sync.dma_start(out=onesA[:], in_=t_ones[:])
        iden = singles.tile([128, 128], bf16)
        nc.sync.dma_start(out=iden[:], in_=t_iden[:])
        c_shift = singles.tile([128, 1], f32)
        nc.vector.memset(c_shift[:], ACT_SHIFT)
        c_hpi = singles.tile([128, 1], f32)
        nc.vector.memset(c_hpi[:], float(np.pi / 2))
        negb2 = singles.tile([3, 1], f32)
        nc.vector.tensor_scalar(out=negb2[:], in0=b2[:], scalar1=-1.0, scalar2=None, op0=OP.mult)

        # ---- prologue: view embedding -> vembT [27, nrays] bf16 ----
        vd = singles.tile([128, 8, 3], f32)
        nc.sync.dma_start(out=vd[:], in_=t_vd[:])
        sq = singles.tile([128, 8, 3], f32)
        nc.vector.tensor_tensor(out=sq[:], in0=vd[:], in1=vd[:], op=OP.mult)
        ss = singles.tile([128, 8], f32)
        nc.vector.reduce_sum(out=ss[:], in_=sq[:], axis=mybir.AxisListType.X)
        nrm = singles.tile([128, 8], f32)
        nc.scalar.activation(out=nrm[:], in_=ss[:], func=AF.Sqrt)
        nc.vector.tensor_scalar(out=nrm[:], in0=nrm[:], scalar1=EPS, scalar2=None, op0=OP.add)
        rinv = singles.tile([128, 8], f32)
        nc.vector.reciprocal(out=rinv[:], in_=nrm[:])
        dn = singles.tile([128, 8, 3], f32)
        nc.vector.tensor_tensor(out=dn[:], in0=vd[:], in1=_ap(rinv[:], [[0, 3]]), op=OP.mult)
        ang = singles.tile([128, 8, 12], f32)
        for k in range(4):
            dstk = _ap(ang[:], dims=[ang[:].ap[0], [12, 8], [4, 3]], offset_add=k)
            nc.vector.tensor_scalar(out=dstk, in0=dn[:], scalar1=float(2 ** k),
                                    scalar2=None, op0=OP.mult)
        V = singles.tile([128, 8, 27], f32)
        nc.vector.tensor_copy(V[:, :, 0:3], dn[:])
        nc.scalar.activation(out=V[:, :, 3:15], in_=ang[:], func=AF.Sin)
        nc.scalar.activation(out=V[:, :, 15:27], in_=ang[:], func=AF.Sin, bias=c_hpi[:])
        vembT = singles.tile([27, 1024], bf16)
        for j in range(8):
            Vj = singles.tile([128, 27], bf16, tag=f"vjc{j}", name=f"vjc{j}")
            nc.vector.tensor_copy(Vj[:], V[:, j, :])
            pv = ps_tr.tile([27, 128], bf16, tag="pt")
            nc.tensor.transpose(out=pv[:], in_=Vj[:], identity=iden[:])
            nc.vector.tensor_copy(vembT[:, j * 128:(j + 1) * 128], pv[:])

        outacc = singles.tile([3, nrays], f32)
        blast = singles.tile([3, nrays], f32)

        # ---- main loop ----
        nsb = (nblk + SB - 1) // SB
        for isb in range(nsb):
            base = isb * SB
            nb = min(SB, nblk - base)
            rp = rp_pool.tile([128, SB, NT, 3], f32, tag="rp")
            nc.sync.dma_start(out=rp[:, 0:nb],
                              in_=t_rp[base:base + nb].rearrange("b p t c -> p b t c"))
            u = sbp.tile([128, SB, NT, 3], f32, tag="u")
            nc.vector.tensor_scalar(out=u[:, 0:nb], in0=rp[:, 0:nb], scalar1=float(G - 1),
                                    scalar2=0.0, op0=OP.mult, op1=OP.max)
            nc.vector.tensor_scalar(out=u[:, 0:nb], in0=u[:, 0:nb], scalar1=float(G - 1),
                                    scalar2=None, op0=OP.min)
            i0 = sbp.tile([128, SB, NT, 3], i32, tag="i0")
            nc.vector.tensor_copy(i0[:, 0:nb], u[:, 0:nb])
            i0f = sbp.tile([128, SB, NT, 3], f32, tag="i0f")
            nc.vector.tensor_copy(i0f[:, 0:nb], i0[:, 0:nb])
            # exact floor fixup: if i0f > u: i0f -= 1  (handles any cast rounding)
            gt = sbp.tile([128, SB, NT, 3], f32, tag="gt")
            nc.vector.tensor_tensor(out=gt[:, 0:nb], in0=i0f[:, 0:nb], in1=u[:, 0:nb], op=OP.is_gt)
            nc.vector.tensor_tensor(out=i0f[:, 0:nb], in0=i0f[:, 0:nb], in1=gt[:, 0:nb], op=OP.subtract)
            nc.vector.tensor_scalar(out=i0f[:, 0:nb], in0=i0f[:, 0:nb], scalar1=float(G - 2),
                                    scalar2=None, op0=OP.min)
            fr = sbp.tile([128, SB, NT, 3], f32, tag="fr")
            nc.vector.tensor_tensor(out=fr[:, 0:nb], in0=u[:, 0:nb], in1=i0f[:, 0:nb], op=OP.subtract)
            frb = sbp.tile([128, SB, NT, 3], bf16, tag="frb")
            nc.vector.tensor_copy(frb[:, 0:nb], fr[:, 0:nb])
            nc.vector.tensor_copy(i0[:, 0:nb], i0f[:, 0:nb])  # exact ints, any rounding ok
            idxt = sbp.tile([128, SB, NT], i32, tag="idxt")
            nc.vector.tensor_scalar(out=idxt[:, 0:nb], in0=i0[:, 0:nb, :, 0],
                                    scalar1=(G - 1) * G, scalar2=None, op0=OP.mult)
            tmpy = sbp.tile([128, SB, NT], i32, tag="tmpy")
            nc.vector.tensor_scalar(out=tmpy[:, 0:nb], in0=i0[:, 0:nb, :, 1],
                                    scalar1=G, scalar2=None, op0=OP.mult)
            nc.vector.tensor_tensor(out=idxt[:, 0:nb], in0=idxt[:, 0:nb], in1=tmpy[:, 0:nb], op=OP.add)
            nc.vector.tensor_tensor(out=idxt[:, 0:nb], in0=idxt[:, 0:nb], in1=i0[:, 0:nb, :, 2], op=OP.add)

            for ib in range(nb):
                b = base + ib
                Gt = gpool.tile([128, NT, 2 * REC], bf16, tag="G")
                for t in range(NT):
                    nc.gpsimd.indirect_dma_start(
                        out=Gt[:, t, :], out_offset=None, in_=t_tab[:],
                        in_offset=bass.IndirectOffsetOnAxis(ap=idxt[:, ib, t:t + 1], axis=0))

                fx = frb[:, ib, :, 0]
                fy = frb[:, ib, :, 1]
                fz = frb[:, ib, :, 2]
                # z-lerp
                zl = lpool.tile([128, NT, REC], bf16, tag="zl")
                nc.vector.tensor_tensor(out=zl[:], in0=Gt[:, :, REC:2 * REC],
                                        in1=Gt[:, :, 0:REC], op=OP.subtract)
                nc.vector.tensor_tensor(out=zl[:], in0=zl[:], in1=_ap(fz, [[0, REC]]), op=OP.mult)
                nc.vector.tensor_tensor(out=zl[:], in0=zl[:], in1=Gt[:, :, 0:REC], op=OP.add)
                # y-lerp on (0:13 | 13:26) and (26:39 | 39:52)
                yl = lpool.tile([128, NT, 2, CH], bf16, tag="yl")
                z0 = zl[:]
                zy0 = _ap(z0, dims=[z0.ap[0], z0.ap[1], [2 * CH, 2], [1, CH]])
                zy1 = _ap(z0, dims=[z0.ap[0], z0.ap[1], [2 * CH, 2], [1, CH]], offset_add=CH)
                fyb = _ap(fy, [[0, 2], [0, CH]])
                nc.vector.tensor_tensor(out=yl[:], in0=zy1, in1=zy0, op=OP.subtract)
                nc.vector.tensor_tensor(out=yl[:], in0=yl[:], in1=fyb, op=OP.mult)
                nc.vector.tensor_tensor(out=yl[:], in0=yl[:], in1=zy0, op=OP.add)
                # x-lerp -> L[:,:,0:13]
                L = lpool.tile([128, NT, 16], bf16, tag="L")
                nc.vector.tensor_tensor(out=L[:, :, 0:CH], in0=yl[:, :, 1, :], in1=yl[:, :, 0, :], op=OP.subtract)
                nc.vector.tensor_tensor(out=L[:, :, 0:CH], in0=L[:, :, 0:CH], in1=_ap(fx, [[0, CH]]), op=OP.mult)
                nc.vector.tensor_tensor(out=L[:, :, 0:CH], in0=L[:, :, 0:CH], in1=yl[:, :, 0, :], op=OP.add)

                # density -> u=exp(d+shift); sp=u(1-u/2); alpha=0.5u(1-0.75u)
                uu = spool.tile([128, NT], f32, tag="uu")
                nc.scalar.activation(out=uu[:], in_=L[:, :, 12], func=AF.Exp, bias=c_shift[:])
                sp_t = spool.tile([128, NT], f32, tag="sp_t")
                nc.vector.tensor_scalar(out=sp_t[:], in0=uu[:], scalar1=-0.5, scalar2=1.0,
                                        op0=OP.mult, op1=OP.add)
                nc.vector.tensor_tensor(out=sp_t[:], in0=sp_t[:], in1=uu[:], op=OP.mult)
                alpha = spool.tile([128, NT], f32, tag="alpha")
                nc.vector.tensor_scalar(out=alpha[:], in0=uu[:], scalar1=-0.375, scalar2=0.5,
                                        op0=OP.mult, op1=OP.add)
                nc.vector.tensor_tensor(out=alpha[:], in0=alpha[:], in1=uu[:], op=OP.mult)

                # cumsum: S_excl(p,2r)=sum_{q<p} sp[q,2r]; odd cols += colsum(even)
                pscomb = ps_cs.tile([128, 2 * NT], f32, tag="cs")
                psS = pscomb[:, 0:NT]
                psC = pscomb[:, NT:2 * NT]
                nc.tensor.matmul(out=psS, lhsT=ustr[:], rhs=sp_t[:], start=True, stop=True)
                nc.tensor.matmul(out=psC, lhsT=onesA[:], rhs=sp_t[:], start=True, stop=True)
                csb = spool.tile([128, NT], f32, tag="csb")
                nc.vector.tensor_copy(csb[:], psC)
                sex = spool.tile([128, NT], f32, tag="sex")
                pse = _ap(psS, dims=[psS.ap[0], [2, NT // 2]])
                pso = _ap(psS, dims=[psS.ap[0], [2, NT // 2]], offset_add=1)
                cse = _ap(csb[:], dims=[csb[:].ap[0], [2, NT // 2]])
                sxe = _ap(sex[:], dims=[sex[:].ap[0], [2, NT // 2]])
                sxo = _ap(sex[:], dims=[sex[:].ap[0], [2, NT // 2]], offset_add=1)
                nc.vector.tensor_copy(sxe, pse)
                nc.vector.tensor_tensor(out=sxo, in0=pso, in1=cse, op=OP.add)
                A_t = spool.tile([128, NT], f32, tag="A_t")
                nc.scalar.activation(out=A_t[:], in_=sex[:], func=AF.Exp, scale=-0.5)
                nc.vector.tensor_tensor(out=L[:, :, 13], in0=alpha[:], in1=A_t[:], op=OP.mult)
                nc.vector.tensor_tensor(out=L[:, :, 14], in0=alpha[:], in1=A_t[:], op=OP.mult)
                nc.vector.tensor_tensor(out=L[:, :, 15], in0=alpha[:], in1=A_t[:], op=OP.mult)
                rtot = spool.tile([3, 8], f32, tag="rtot")
                cb3 = csb[0:3, :]
                ce3 = _ap(cb3, dims=[[cb3.ap[0][0], 3], [2, NT // 2]])
                co3 = _ap(cb3, dims=[[cb3.ap[0][0], 3], [2, NT // 2]], offset_add=1)
                nc.vector.tensor_tensor(out=rtot[:], in0=ce3, in1=co3, op=OP.add)
                nc.scalar.activation(out=blast[:, b * 8:(b + 1) * 8], in_=rtot[:], func=AF.Exp, scale=-0.5)

                # transpose -> XT [16, BLK]
                XT = xpool.tile([16, BLK], bf16, tag="XT")
                wT3 = xpool.tile([3, BLK], bf16, tag="wT3")
                for t in range(NT):
                    pt = ps_tr.tile([16, 128], bf16, tag="pt")
                    nc.tensor.transpose(out=pt[:], in_=L[:, t, :], identity=iden[:])
                    nc.vector.tensor_copy(XT[:, t * 128:(t + 1) * 128], pt[:])
                    pw = ps_tr.tile([3, 128], bf16, tag="pw")
                    nc.tensor.transpose(out=pw[:], in_=L[:, t, 13:16], identity=iden[:])
                    nc.vector.tensor_copy(wT3[:, t * 128:(t + 1) * 128], pw[:])

                vb = xpool.tile([27, BLK], bf16, tag="vb")
                vsl = vembT[:, b * 8:(b + 1) * 8]
                nc.vector.tensor_copy(vb[:].rearrange("p (r k) -> p r k", r=8),
                                      _ap(vsl, [[0, 256]]))

                rgbblk = hpool.tile([3, BLK], bf16, tag="rgbblk")
                for c in range(4):
                    cs = slice(c * 512, (c + 1) * 512)
                    p0 = ps_mlp.tile([128, 512], f32, tag="p0")
                    nc.tensor.matmul(out=p0[:], lhsT=w0a[:], rhs=XT[0:CF, cs], start=True, stop=False)
                    nc.tensor.matmul(out=p0[:], lhsT=w0b[:], rhs=vb[:, cs], start=False, stop=True)
                    h0 = hpool.tile([128, 512], bf16, tag="h0")
                    nc.scalar.activation(out=h0[:], in_=p0[:], func=AF.Relu, bias=b0[:])
                    p1 = ps_mlp.tile([128, 512], f32, tag="p1")
                    nc.tensor.matmul(out=p1[:], lhsT=w1[:], rhs=h0[:], start=True, stop=True)
                    h1 = hpool.tile([128, 512], bf16, tag="h1")
                    nc.scalar.activation(out=h1[:], in_=p1[:], func=AF.Relu, bias=b1[:])
                    p2 = ps_mlp.tile([3, 512], f32, tag="p2")
                    nc.tensor.matmul(out=p2[:], lhsT=w2[:], rhs=h1[:], start=True, stop=True)
                    nc.scalar.activation(out=rgbblk[:, cs], in_=p2[:], func=AF.Sigmoid, bias=b2[:])
                wr = hpool.tile([3, BLK], bf16, tag="wr")
                nc.vector.tensor_tensor(out=wr[:], in0=rgbblk[:], in1=wT3[:], op=OP.mult)
                ww = wr[:]
                rseg = _ap(ww, dims=[ww.ap[0], [256, 8], [1, 256]])
                nc.vector.reduce_sum(out=outacc[:, b * 8: b * 8 + 8],
                                     in_=rseg, axis=mybir.AxisListType.X)

        nc.vector.tensor_tensor(out=outacc[:], in0=outacc[:], in1=blast[:], op=OP.add)
        nc.sync.dma_start(out=t_out[:], in_=outacc[:])

    nc.compile()
    return nc


def _host_prep(inputs, nblk):
    table = build_table(inputs["density_grid"], inputs["k0"])
    w0 = np.asarray(inputs["w0"])
    consts = {
        "tab": table,
        "w0a": np.ascontiguousarray(w0[0:CF]).astype(BF16),
        "w0b": np.ascontiguousarray(w0[CF:CF + 27]).astype(BF16),
        "w1": np.asarray(inputs["w1"]).astype(BF16),
        "w2": np.asarray(inputs["w2"]).astype(BF16),
        "b0": np.asarray(inputs["b0"]).astype(np.float32).reshape(128, 1),
        "b1": np.asarray(inputs["b1"]).astype(np.float32).reshape(128, 1),
        "b2": np.asarray(inputs["b2"]).astype(np.float32).reshape(3, 1),
        "ustr": np.triu(np.ones((128, 128), np.float32), 1),  # [q,p]=1 if q<p
        "onesA": np.ones((128, 128), np.float32),
        "iden": np.eye(128, dtype=np.float32).astype(BF16),
    }
    rp_all = np.asarray(inputs["ray_pts"], np.float32)
    vd_all = np.asarray(inputs["viewdirs"], np.float32)
    in_maps = []
    for c in range(NCORES):
        rp = rp_all[c * RAYS_PER_CORE:(c + 1) * RAYS_PER_CORE].reshape(-1, 3)
        rp = rp[: nblk * BLK]
        rp = rp.reshape(nblk, NT, 128, 3).transpose(0, 2, 1, 3)  # sample = t*128+p
        vd = vd_all[c * RAYS_PER_CORE:(c + 1) * RAYS_PER_CORE]
        vd = vd.reshape(8, 128, 3).transpose(1, 0, 2)            # ray = j*128+p
        in_maps.append(dict(consts, rp=np.ascontiguousarray(rp), vd=np.ascontiguousarray(vd)))
    return in_maps


def run(inputs, nblk=NBLK_FULL, profile=False):
    if nblk not in _cache:
        _cache[nblk] = build_nc(nblk)
    nc = _cache[nblk]
    in_maps = _host_prep(inputs, nblk)
    if profile:
        from bench_lib import run_and_profile
        results, exec_ns = run_and_profile(nc, in_maps, profile=True)
        return results, exec_ns
    res = run_bass_kernel_spmd(nc, in_maps, core_ids=list(range(NCORES)))
    return res.results, None


def kernel(**inputs):
    results, _ = run(inputs, NBLK_FULL)
    out = np.concatenate([r["out"].T for r in results], axis=0)
    return out.astype(np.float32)
